# revision 13
# baseline (speedup 1.0000x reference)
"""CountVectorizer-as-embedding-bag Trainium2 kernel (bf16 2-row blocks).

Computes out[b, :] = sum_s W[token_ids[b, s], :] + bias  (== counts @ W + b
without materializing the [B, V] counts matrix).

Sharding: data-parallel over batch across 8 NeuronCores (128 rows/core).

The gather is SDMA-packet-bound (~2.6 ns/packet aggregate across the 4
SWDGE queues), so the kernel minimizes packet count:
  - W is repacked host-side as W2[50000, 256] bf16 — each 512B row holds
    TWO adjacent vocab rows. One gather packet serves every token hitting
    either row of the block (per-core block dedup: ~23.0k packets instead
    of 25.6k+pad single-row packets). Block indices fit int16 per
    half-vocab, so there are 2 halves x 6 sub-gathers, sizes picked so
    each of the 4 SWDGE queues carries exactly CAPH/2 packets.
  - Every gathered slot is real W data (pads repeat the last block id,
    staying in the same HBM row buffer), so unused sel lanes multiply
    finite values, never garbage.
  - sel is a full-width one-hot [128 slots, 128 batch rows] per virtual
    chunk (vc = 2*chunk + sub-row): sel[p, vc, r] = (rr[p, vc] == r),
    built on DVE in bf16 into a 3-deep rotating pool, 16 vchunks per
    is_equal. One PE matmul per vchunk accumulates into a single
    [128, 128] fp32 PSUM tile (no 32-row grouping / tile_position).
  - Bias is a K=1 ones^T @ b matmul with start=True that initializes all
    PSUM cells.
bf16 W rounding gives ~2e-3 relative error, well under the 2e-2 gate.
"""

import numpy as np
import ml_dtypes

import concourse.bacc as bacc
import concourse.mybir as mybir
import concourse.tile as tile
from concourse.bass_utils import run_bass_kernel_spmd

B, S, V, D = 1024, 200, 100000, 128
N_CORES = 8
P = 128
BP = B // N_CORES        # 128 batch rows per core
NBLK = V // 2            # 50000 two-row blocks
HBLK = NBLK // 2         # 25000 blocks per half (int16-indexable)

CAPH = 11776             # slots per half (92 chunks); seed-0 max is 11628
NCH = 2 * CAPH // P      # 184 chunks total
NVC = 2 * NCH            # 368 virtual chunks (2 sub-rows per block)
IDC = 2 * CAPH // 16     # int16 idx columns = 1472
SELB = 16                # vchunks per sel build / pool buffer
assert NVC % SELB == 0

# per-half sub-gather chunk counts; emission interleaves halves and
# round-robins queues so each queue gets exactly (CAPH*2)/4 packets.
_SUBS = [15, 15, 15, 15, 16, 16]
assert sum(_SUBS) * P == CAPH

_CACHE: dict = {}


def _build_nc():
    nc = bacc.Bacc(
        "TRN2",
        target_bir_lowering=False,
        debug=False,
        num_devices=N_CORES,
        num_swdge_queues=4,
        dynamic_dma_scratch_size=49152,
    )
    f32 = mybir.dt.float32
    bf16 = mybir.dt.bfloat16
    ids = nc.dram_tensor("ids", [P, IDC], mybir.dt.int16, kind="ExternalInput")
    rr = nc.dram_tensor("rr", [P, NVC], bf16, kind="ExternalInput")
    iota = nc.dram_tensor("iota", [P, SELB * P], bf16, kind="ExternalInput")
    # W2 bytes are bf16 pairs, but typed f32 so the gather takes the
    # (much faster) 4-byte ucode path; the PE reads a bf16 bitcast view.
    W2 = nc.dram_tensor("W2", [NBLK, D], f32, kind="ExternalInput")
    bvec = nc.dram_tensor("bvec", [1, D], f32, kind="ExternalInput")
    out = nc.dram_tensor("out", [P, D], f32, kind="ExternalOutput")

    with tile.TileContext(nc) as tc:
        with (
            tc.tile_pool(name="const", bufs=1) as cpool,
            tc.tile_pool(name="selp", bufs=3) as spool,
            tc.tile_pool(name="psum", bufs=1, space="PSUM") as ppool,
        ):
            ids_sb = cpool.tile([P, IDC], mybir.dt.int16)
            rr_sb = cpool.tile([P, NVC], bf16)
            iota_sb = cpool.tile([P, SELB * P], bf16)
            b_sb = cpool.tile([1, D], f32)
            ones_sb = cpool.tile([1, P], f32)
            acc_sb = cpool.tile([P, D], f32)
            G = cpool.tile([P, NCH * D], f32)        # all gather payloads

            # ids first on the SP queue: the gathers' only input dependency.
            nc.sync.dma_start(out=ids_sb[:], in_=ids[:])
            # aux loads on the Activation HW-DGE queue (don't delay ids).
            nc.scalar.dma_start(out=rr_sb[:], in_=rr[:])
            nc.scalar.dma_start(out=iota_sb[:], in_=iota[:])
            nc.scalar.dma_start(out=b_sb[:], in_=bvec[:])
            nc.vector.memset(ones_sb[:], 1.0)

            # gathers: interleave halves, round-robin queues
            order = [(h, s) for s in range(len(_SUBS)) for h in range(2)]
            sub_ch0 = {}
            c = 0
            for h in range(2):
                for s, nch in enumerate(_SUBS):
                    sub_ch0[(h, s)] = c
                    c += nch
            qn = 0
            for h, s in order:
                nch = _SUBS[s]
                nidx = nch * P
                c0 = sub_ch0[(h, s)]
                nc.gpsimd.dma_gather(
                    G[:, c0 * D : (c0 + nch) * D].rearrange(
                        "p (c e) -> p c e", e=D
                    ),
                    W2[h * HBLK : (h + 1) * HBLK],
                    ids_sb[:, c0 * 8 : (c0 + nch) * 8],
                    nidx,
                    nidx,
                    D,
                    single_packet=False,
                    queue_num=qn,
                )
                qn = (qn + 1) % 4
            # bf16 view: [P, NVC, D] virtual chunks (2 sub-rows per block)
            Gv = G[:].bitcast(bf16).rearrange("p (v e) -> p v e", e=D)

            psum = ppool.tile([P, D], f32)
            # Broadcast bias to every output row; start=True sets has_written
            # on all PSUM cells so everything below accumulates.
            nc.tensor.matmul(
                out=psum[:],
                lhsT=ones_sb[:],
                rhs=b_sb[:],
                start=True,
                stop=False,
                skip_group_check=True,
            )

            for bb in range(NVC // SELB):
                sel = spool.tile([P, SELB * P], bf16, tag="sel")
                nc.vector.tensor_tensor(
                    out=sel[:].rearrange("p (j c) -> p j c", c=P),
                    in0=rr_sb[:, bb * SELB : (bb + 1) * SELB].to_broadcast(
                        [P, SELB, P]
                    ),
                    in1=iota_sb[:].rearrange("p (j c) -> p j c", c=P),
                    op=mybir.AluOpType.is_equal,
                )
                for j in range(SELB):
                    vc = bb * SELB + j
                    nc.tensor.matmul(
                        out=psum[:],
                        lhsT=sel[:, j * P : (j + 1) * P],
                        rhs=Gv[:, vc, :],
                        start=False,
                        stop=(vc == NVC - 1),
                        skip_group_check=True,
                    )

            nc.vector.tensor_copy(out=acc_sb[:], in_=psum[:])
            nc.scalar.dma_start(out=out[:], in_=acc_sb[:])

    nc.compile()
    return nc


def _get_nc():
    if "nc" not in _CACHE:
        _CACHE["nc"] = _build_nc()
    return _CACHE["nc"]


def _core_inputs(shard: np.ndarray):
    """shard: [128, 200] int32 -> (ids [128, IDC] int16, rr [128, NVC] f32).

    Raises ValueError on capacity overflow (caller falls back to numpy).
    """
    v = shard.reshape(-1).astype(np.int64)
    r = np.repeat(np.arange(BP, dtype=np.int64), S)
    beta = v >> 1
    sub = v & 1

    ids_halves = []
    rr_mat = np.full((NVC, P), -1.0, dtype=ml_dtypes.bfloat16)
    for h in range(2):
        m = (beta >= h * HBLK) & (beta < (h + 1) * HBLK)
        rel = beta[m] - h * HBLK
        jh = sub[m]
        rh = r[m]
        order = np.lexsort((rh, jh, rel))
        rel_s, j_s, r_s = rel[order], jh[order], rh[order]
        key = rel_s * 2 + j_s
        change = np.r_[True, key[1:] != key[:-1]]
        startpos = np.flatnonzero(change)
        grp = np.cumsum(change) - 1
        k = np.arange(key.size) - startpos[grp]        # occurrence rank
        cnt = np.diff(np.r_[startpos, key.size])       # per (block, sub)
        ublk = key[startpos] >> 1
        inst = np.zeros(HBLK, dtype=np.int64)
        np.maximum.at(inst, ublk, cnt)                 # instances per block
        total = int(inst.sum())
        if total > CAPH:
            raise ValueError(f"half {h} overflow: {total} > {CAPH}")
        hit = np.flatnonzero(inst)
        blocks_list = np.repeat(hit, inst[hit])        # slot -> block id
        ids_pad = np.full(
            CAPH, blocks_list[-1] if total else 0, dtype=np.int16
        )
        ids_pad[:total] = blocks_list.astype(np.int16)
        ids_halves.append(ids_pad)
        off = np.zeros(HBLK, dtype=np.int64)
        off[1:] = np.cumsum(inst)[:-1]                 # slot base per block
        slot = off[rel_s] + k + h * CAPH               # global slot
        vc = 2 * (slot // P) + j_s
        rr_mat[vc, slot % P] = r_s.astype(ml_dtypes.bfloat16)

    ids_all = np.concatenate(ids_halves)               # [2*CAPH]
    ids_in = np.ascontiguousarray(
        np.tile(ids_all.reshape(-1, 16).T, (8, 1))
    )                                                  # [128, IDC]
    rr_in = np.ascontiguousarray(rr_mat.T)             # [128, NVC]
    assert ids_in.shape == (P, IDC) and rr_in.shape == (P, NVC)
    return ids_in, rr_in


def _kernel_numpy(token_ids, W, b):
    out = np.tile(b.astype(np.float32), (B, 1))
    for i in range(B):
        out[i] += W[token_ids[i]].sum(axis=0)
    return out.astype(np.float32)


def _make_in_maps(inputs):
    token_ids = np.ascontiguousarray(
        np.asarray(inputs["token_ids"], dtype=np.int32)
    )
    W = np.ascontiguousarray(np.asarray(inputs["W"], dtype=np.float32))
    b = np.ascontiguousarray(np.asarray(inputs["b"], dtype=np.float32))
    b2 = np.ascontiguousarray(b.reshape(1, D))
    W2 = np.ascontiguousarray(
        W.astype(ml_dtypes.bfloat16).reshape(NBLK, 2 * D).view(np.float32)
    )
    iota = np.ascontiguousarray(
        np.tile(
            np.tile(
                np.arange(P, dtype=np.float32).astype(ml_dtypes.bfloat16),
                SELB,
            )[None, :],
            (P, 1),
        )
    )
    in_maps = []
    for c in range(N_CORES):
        ids_in, rr_in = _core_inputs(token_ids[c * BP : (c + 1) * BP])
        in_maps.append(
            {"ids": ids_in, "rr": rr_in, "iota": iota, "W2": W2, "bvec": b2}
        )
    return in_maps


def kernel(token_ids, W, b, **kwargs):
    token_ids = np.ascontiguousarray(np.asarray(token_ids, dtype=np.int32))
    W = np.ascontiguousarray(np.asarray(W, dtype=np.float32))
    b = np.ascontiguousarray(np.asarray(b, dtype=np.float32))
    assert token_ids.shape == (B, S) and W.shape == (V, D) and b.shape == (D,)

    try:
        in_maps = _make_in_maps({"token_ids": token_ids, "W": W, "b": b})
    except ValueError:
        # capacity overflow on unexpected data: slow-but-correct path
        return _kernel_numpy(token_ids, W, b)

    nc = _get_nc()
    res = run_bass_kernel_spmd(nc, in_maps, core_ids=list(range(N_CORES)))
    return np.concatenate(
        [res.results[c]["out"] for c in range(N_CORES)], axis=0
    ).astype(np.float32)
